# revision 4
# baseline (speedup 1.0000x reference)
"""LAINet forward (nn_LAINetOriginal) as a Bass/Tile kernel on 8 NeuronCores.

Sharding: 1000 windows split 8 x 125; each core also recomputes a 37/38-window
reflect-mapped halo so the Conv2d smoother needs no cross-core communication
(LWIN = 200 local windows per core). BatchNorm stats are over the batch axis,
which is fully local under window sharding, so numerics match the reference.

Data path: x and all weights are cast to bf16 on host (fp32 PSUM accumulation
on device); BN stats / softmax / conv accumulate in fp32.

Per-core device pipeline (all 200 windows, groups of 4 packed into the
128-partition dim as 4 x 32):
  L1   x[win] @ W1[win] -> PSUM[4x32, 128]  (K=125 x 4 chunks, col-grouped)
  ReLU+b1 (ACT, PSUM->SBUF), BN over batch (bn_stats/bn_aggr + tensor_scalar)
  L2   -> PSUM o[4x32, 128] (K=30, diagonal tile_position packing), +b2
  softmax over the 7 ancestries: exp (ACT), partition-sums via matmul with a
  0/1 selector, reciprocal, broadcast-back via matmul, multiply
  p scattered (DMA) into two [7, y*64+b] layouts (per channel), then
  replicated into shifted form P_rep[(t,i), y'] = p[i, y'+t] (16 DMAs each)
  Conv = 10 accumulating matmuls per output chunk over K=(i,t) (=112/77),
  2 output col groups (the two haplotype columns), + conv bias, DMA out.
"""
import numpy as np

B = 64
B2 = 128
INPUT_DIM = 500000
WIN = 500
N_WIN = 1000
HID = 30
ANC = 7
KS = 75
EPS = 1e-5
NCORES = 8
OWN = N_WIN // NCORES          # 125
HALO = KS // 2                 # 37
LWIN = 200                     # 37 left halo + 125 own + 38 right halo
NG = LWIN // 4                 # 50 groups of 4 windows
KCH = 4                        # contraction chunks of 125 for L1
KW = WIN // KCH                # 125
TPC = 16                       # conv taps per contraction chunk
NKC = 5                        # conv contraction chunks (16,16,16,16,11)
PCF = 214 * 64                 # p_c free size (y padded to 214, b=64)
PRF = 199 * 64                 # P_rep free size
YC = 8                         # conv output windows per matmul (N = 8*64 = 512)


def _core_windows(k):
    idx = []
    for i in range(OWN * k - HALO, OWN * (k + 1) + HALO + 1):
        if i < 0:
            i = -i
        elif i > N_WIN - 1:
            i = 2 * (N_WIN - 1) - i
        idx.append(i)
    return np.asarray(idx, dtype=np.int64)


def _build_program():
    import concourse.bacc as bacc
    import concourse.tile as tile
    from concourse import mybir

    BF16 = mybir.dt.bfloat16
    F32 = mybir.dt.float32
    AF = mybir.ActivationFunctionType
    ALU = mybir.AluOpType

    nc = bacc.Bacc('TRN2', target_bir_lowering=False, debug=False,
                   num_devices=1)
    xs = nc.dram_tensor('xs', [LWIN, KW, KCH, B2], BF16, kind='ExternalInput')
    w1 = nc.dram_tensor('w1', [LWIN, KW, KCH, 32], BF16, kind='ExternalInput')
    b1p = nc.dram_tensor('b1p', [NG, 128], F32, kind='ExternalInput')
    w2p = nc.dram_tensor('w2p', [NG, 128, 32], BF16, kind='ExternalInput')
    b2p = nc.dram_tensor('b2p', [NG, 128], F32, kind='ExternalInput')
    stm = nc.dram_tensor('stm', [128, 4], F32, kind='ExternalInput')
    sbm = nc.dram_tensor('sbm', [4, 128], F32, kind='ExternalInput')
    cwp = nc.dram_tensor('cwp', [2, NKC, 112, 7], BF16, kind='ExternalInput')
    cbp = nc.dram_tensor('cbp', [7, 1], F32, kind='ExternalInput')
    ob = nc.dram_tensor('ob', [NG, 128, 128], F32, kind='ExternalOutput')
    osm = nc.dram_tensor('osm', [2, 7, OWN, B], F32, kind='ExternalOutput')

    with tile.TileContext(nc) as tc:
        with tc.tile_pool(name='per', bufs=1) as per, \
             tc.tile_pool(name='io', bufs=3) as io, \
             tc.tile_pool(name='mid', bufs=3) as mid, \
             tc.tile_pool(name='st', bufs=4) as stp, \
             tc.tile_pool(name='ps1p', bufs=2, space='PSUM') as ps1p, \
             tc.tile_pool(name='ps2p', bufs=2, space='PSUM') as ps2p, \
             tc.tile_pool(name='pssp', bufs=1, space='PSUM') as pssp, \
             tc.tile_pool(name='psbp', bufs=1, space='PSUM') as psbp, \
             tc.tile_pool(name='pscp', bufs=2, space='PSUM') as pscp:
            # persistent tensors
            p0c = per.tile([7, PCF], BF16, tag='p0c')
            p1c = per.tile([7, PCF], BF16, tag='p1c')
            p0r = per.tile([112, PRF], BF16, tag='p0r')
            p1r = per.tile([112, PRF], BF16, tag='p1r')
            stt = per.tile([128, 4], F32, tag='stt')
            sbt = per.tile([4, 128], F32, tag='sbt')
            cwt = per.tile([112, 2, NKC, 7], BF16, tag='cwt')
            cbt = per.tile([7, 1], F32, tag='cbt')
            epst = per.tile([128, 1], F32, tag='epst')
            nc.vector.memset(epst, float(EPS))
            nc.sync.dma_start(out=stt, in_=stm.ap())
            nc.sync.dma_start(out=sbt, in_=sbm.ap())
            nc.sync.dma_start(out=cwt, in_=cwp.ap().rearrange('w k p o -> p w k o'))
            nc.sync.dma_start(out=cbt, in_=cbp.ap())
            # zero the pad tail of p_c (y = 200..213) so replicated reads are finite
            nc.vector.memset(p0c[:, 200 * 64:], 0.0)
            nc.vector.memset(p1c[:, 200 * 64:], 0.0)

            xs_r = xs.ap().rearrange('(g n) wi ch b -> g wi n ch b', n=4)
            w1_r = w1.ap().rearrange('(g n) wi ch h -> g wi n ch h', n=4)
            for g in range(NG):
                xt = io.tile([KW, 4, KCH, B2], BF16, tag='xt')
                nc.sync.dma_start(out=xt, in_=xs_r[g])
                w1t = io.tile([KW, 4, KCH, 32], BF16, tag='w1t')
                nc.sync.dma_start(out=w1t, in_=w1_r[g])
                b1t = stp.tile([128, 1], F32, tag='b1t')
                nc.sync.dma_start(out=b1t, in_=b1p.ap()[g].rearrange('(p o) -> p o', o=1))
                w2t = stp.tile([128, 32], BF16, tag='w2t')
                nc.sync.dma_start(out=w2t, in_=w2p.ap()[g])
                b2t = stp.tile([128, 1], F32, tag='b2t')
                nc.sync.dma_start(out=b2t, in_=b2p.ap()[g].rearrange('(p o) -> p o', o=1))

                ps1 = ps1p.tile([128, B2], F32, tag='ps1')
                for j in range(4):
                    for ch in range(KCH):
                        nc.tensor.matmul(
                            out=ps1[32 * j:32 * j + 32, :],
                            lhsT=w1t[:, j, ch, :], rhs=xt[:, j, ch, :],
                            start=(ch == 0), stop=(ch == KCH - 1),
                            tile_position=(0, 32 * j))
                ht = mid.tile([128, B2], F32, tag='ht')
                nc.scalar.activation(out=ht, in_=ps1[0:128, :], func=AF.Relu,
                                     bias=b1t, scale=1.0)
                # BN over batch, separately per haplotype channel (free halves)
                hn = mid.tile([128, B2], BF16, tag='hn')
                for c in range(2):
                    stat = stp.tile([128, 6], F32, tag='stat%d' % c)
                    nc.vector.bn_stats(out=stat, in_=ht[:, 64 * c:64 * c + 64])
                    mv = stp.tile([128, 2], F32, tag='mv%d' % c)
                    nc.vector.bn_aggr(out=mv, in_=stat)
                    nc.scalar.activation(out=mv[:, 1:2], in_=mv[:, 1:2],
                                         func=AF.Sqrt, bias=epst, scale=1.0)
                    nc.vector.reciprocal(out=mv[:, 1:2], in_=mv[:, 1:2])
                    nc.vector.tensor_scalar(
                        out=hn[:, 64 * c:64 * c + 64],
                        in0=ht[:, 64 * c:64 * c + 64],
                        scalar1=mv[:, 0:1], scalar2=mv[:, 1:2],
                        op0=ALU.subtract, op1=ALU.mult)
                ps2 = ps2p.tile([128, B2], F32, tag='ps2')
                for j in range(4):
                    nc.tensor.matmul(
                        out=ps2[32 * j:32 * j + 32, :],
                        lhsT=w2t[32 * j:32 * j + 30, :],
                        rhs=hn[32 * j:32 * j + 30, :],
                        start=True, stop=True,
                        tile_position=(32 * j, 32 * j))
                obt = mid.tile([128, B2], F32, tag='obt')
                nc.vector.tensor_scalar(out=obt, in0=ps2[0:128, :], scalar1=b2t,
                                        scalar2=None, op0=ALU.add)
                nc.sync.dma_start(out=ob.ap()[g], in_=obt)
                et = mid.tile([128, B2], F32, tag='et')
                nc.scalar.activation(out=et, in_=obt, func=AF.Exp)
                pss = pssp.tile([4, B2], F32, tag='pss')
                nc.tensor.matmul(out=pss, lhsT=stt, rhs=et, start=True, stop=True)
                rec = stp.tile([4, B2], F32, tag='rec')
                nc.vector.reciprocal(out=rec, in_=pss[0:4, :])
                psb = psbp.tile([128, B2], F32, tag='psb')
                nc.tensor.matmul(out=psb, lhsT=sbt, rhs=rec, start=True, stop=True)
                pt = mid.tile([128, B2], BF16, tag='pt')
                nc.vector.tensor_mul(out=pt, in0=et, in1=psb[0:128, :])
                # scatter p into [a, y*64+b] per channel
                for j in range(4):
                    y = 4 * g + j
                    nc.sync.dma_start(out=p0c[:, y * 64:y * 64 + 64],
                                      in_=pt[32 * j:32 * j + 7, 0:64])
                    nc.sync.dma_start(out=p1c[:, y * 64:y * 64 + 64],
                                      in_=pt[32 * j:32 * j + 7, 64:128])

            # replicate p into shifted layout: P_rep[7t+i, y*64+b] = p[i, (y+t)*64+b]
            for t in range(TPC):
                nc.sync.dma_start(out=p0r[7 * t:7 * t + 7, :],
                                  in_=p0c[:, 64 * t:64 * t + PRF])
                nc.sync.dma_start(out=p1r[7 * t:7 * t + 7, :],
                                  in_=p1c[:, 64 * t:64 * t + PRF])

            # conv: out[cp] = conv(cw[...,0], p_{1-cp}) + conv(cw[...,1], p_{cp})
            p0r3 = p0r.rearrange('p (y b) -> p y b', b=64)
            p1r3 = p1r.rearrange('p (y b) -> p y b', b=64)
            nyc = (OWN + YC - 1) // YC
            for yc in range(nyc):
                y0 = YC * yc
                cnt = min(YC, OWN - y0)
                psc = pscp.tile([39, YC * 64], F32, tag='psc')
                for kc in range(NKC):
                    t0 = TPC * kc
                    np_ = 112 if kc < NKC - 1 else 7 * (KS - TPC * (NKC - 1))
                    for cp in range(2):
                        rhs_a = p1r3 if cp == 0 else p0r3
                        rhs_b = p0r3 if cp == 0 else p1r3
                        nc.tensor.matmul(
                            out=psc[32 * cp:32 * cp + 7, 0:cnt * 64],
                            lhsT=cwt[0:np_, 0, kc, :],
                            rhs=rhs_a[0:np_, y0 + t0:y0 + t0 + cnt, :],
                            start=(kc == 0), stop=False,
                            tile_position=(0, 32 * cp))
                        nc.tensor.matmul(
                            out=psc[32 * cp:32 * cp + 7, 0:cnt * 64],
                            lhsT=cwt[0:np_, 1, kc, :],
                            rhs=rhs_b[0:np_, y0 + t0:y0 + t0 + cnt, :],
                            start=False, stop=(kc == NKC - 1),
                            tile_position=(0, 32 * cp))
                for cp in range(2):
                    ot = stp.tile([7, YC * 64], F32, tag='ot%d' % cp)
                    nc.vector.tensor_scalar(
                        out=ot[:, 0:cnt * 64],
                        in0=psc[32 * cp:32 * cp + 7, 0:cnt * 64],
                        scalar1=cbt, scalar2=None, op0=ALU.add)
                    nc.sync.dma_start(
                        out=osm.ap()[cp, :, y0:y0 + cnt, :],
                        in_=ot[:, 0:cnt * 64].rearrange('p (y b) -> p y b', b=64))
    nc.compile()
    return nc


_NC = None


def _get_nc():
    global _NC
    if _NC is None:
        _NC = _build_program()
    return _NC


def _prep_in_maps(x, W1, b1, W2, b2, conv_w, conv_b):
    import ml_dtypes
    bf16 = ml_dtypes.bfloat16
    x = np.asarray(x, np.float32)
    W1 = np.asarray(W1, np.float32)
    b1 = np.asarray(b1, np.float32)
    W2 = np.asarray(W2, np.float32)
    b2 = np.asarray(b2, np.float32)
    conv_w = np.asarray(conv_w, np.float32)
    conv_b = np.asarray(conv_b, np.float32)

    # x -> [win, wi, ch, c, b] bf16, scaled to [-1, 1]
    xr = x.reshape(B, N_WIN, KCH, KW, 2)
    xg = np.ascontiguousarray(np.transpose(xr, (1, 3, 2, 4, 0)))
    xg = (xg * 2.0 - 1.0).astype(bf16)          # [1000, 125, 4, 2, 64]

    w1g = np.zeros((N_WIN, KW, KCH, 32), np.float32)
    w1g[:, :, :, :30] = np.transpose(W1.reshape(N_WIN, KCH, KW, HID),
                                     (0, 2, 1, 3))
    w1g = w1g.astype(bf16)

    st = np.zeros((128, 4), np.float32)
    sb = np.zeros((4, 128), np.float32)
    for j in range(4):
        st[32 * j:32 * j + 7, j] = 1.0
        sb[j, 32 * j:32 * j + 7] = 1.0

    cw = np.zeros((2, NKC, 112, 7), np.float32)
    for kc in range(NKC):
        t0 = TPC * kc
        tn = min(TPC, KS - t0)
        # cw[w, kc, t*7+i, o] = conv_w[o, i, t0+t, w]
        blk = np.transpose(conv_w[:, :, t0:t0 + tn, :], (3, 2, 1, 0))
        cw[:, kc, :7 * tn, :] = blk.reshape(2, tn * ANC, ANC)
    cwb = cw.astype(bf16)
    cb = conv_b.reshape(7, 1).astype(np.float32)

    in_maps = []
    for k in range(NCORES):
        idx = _core_windows(k)
        xk = np.ascontiguousarray(xg[idx]).reshape(LWIN, KW, KCH, B2)
        w1k = np.ascontiguousarray(w1g[idx])
        b1k = np.zeros((NG, 4, 32), np.float32)
        b1k[:, :, :30] = b1[idx].reshape(NG, 4, HID)
        w2k = np.zeros((NG, 4, 32, 32), np.float32)
        w2k[:, :, :30, :7] = W2[idx].reshape(NG, 4, HID, ANC)
        b2k = np.zeros((NG, 4, 32), np.float32)
        b2k[:, :, :7] = b2[idx].reshape(NG, 4, ANC)
        in_maps.append({
            'xs': xk, 'w1': w1k,
            'b1p': b1k.reshape(NG, 128),
            'w2p': w2k.reshape(NG, 128, 32).astype(bf16),
            'b2p': b2k.reshape(NG, 128),
            'stm': st, 'sbm': sb, 'cwp': cwb, 'cbp': cb,
        })
    return in_maps


def _assemble(results):
    ob_full = np.empty((B, ANC, N_WIN, 2), np.float32)
    os_full = np.empty((B, ANC, N_WIN, 2), np.float32)
    for k in range(NCORES):
        obk = np.asarray(results[k]['ob'], np.float32)
        arr = obk.reshape(NG, 4, 32, 2, B)[:, :, :ANC]
        arr = arr.reshape(LWIN, ANC, 2, B)[HALO:HALO + OWN]
        ob_full[:, :, OWN * k:OWN * (k + 1), :] = np.transpose(
            arr, (3, 1, 0, 2))
        osk = np.asarray(results[k]['osm'], np.float32)  # [2, 7, 125, 64]
        os_full[:, :, OWN * k:OWN * (k + 1), :] = np.transpose(
            osk, (3, 1, 2, 0))
    return ob_full, os_full


def kernel(x, W1, b1, W2, b2, conv_w, conv_b):
    from concourse.bass_utils import run_bass_kernel_spmd
    nc = _get_nc()
    in_maps = _prep_in_maps(x, W1, b1, W2, b2, conv_w, conv_b)
    res = run_bass_kernel_spmd(nc, in_maps, list(range(NCORES)))
    return _assemble(res.results)
